# revision 17
# baseline (speedup 1.0000x reference)
"""Gated multi-head attention (RMSNorm + RoPE + SDPA + sigmoid head gates + out-proj)
as a Trainium2 Bass/Tile kernel, data-parallel over batch across 8 NeuronCores.

Problem shapes (hardcoded): b=8, n=1024, dim=512, heads=8, dim_head=64, theta=1e4.
Each core processes one batch element; no collectives needed.

Per-core dataflow (bf16 matmul operands, fp32 PSUM accumulation):
  x [1024,512] --RMSnorm--> xhat(bf16) --PE transpose--> xhatT [512dim, 1024n]
  qT,kT = (w_qkv*gamma)^T-slices @ xhatT  (psum [128(2 heads x 64d), n])
  RoPE in transposed layout: q*cosT + stream_shuffle(q)*sinT_signed (bf16)
  v natural [n,512] -> v_aug lhsT blocks per head: 64 v cols + 64 ones cols
  (even heads v first, odd heads ones first so AV lands on home partitions)
  per head pair mt:
    S^T[j,i] = kT-slice @ qT-slice (K=64, one 1024-wide mm per (jc,hh),
    hh row-split on the PE), exp on ACT -> es[jc][hh] bf16 in SBUF;
    AV^T accumulated over j per 512-wide i chunk; ones columns give a
    replicated softmax denominator on the opposite partition half
  denominator rows -> denomT[8, n] via SBUF DMA; gate folded as
  eff_denom = denom * (1 + exp(-(xn@w_g + b_g))) so gate+normalize is one
  reciprocal: cT = 1/eff_denom (no Sigmoid table set on ACT)
  cb = E-matmul broadcast of cT to head partitions (bf16 psum, read directly
  by the avg multiplies); avg = avraw * cb; out[i,:] = sum_mt avg @ w_o-slice

Scheduling: the exp stream on ACT (64 x [128,1024]) is the pacing resource.
Pair-0 scores are issued as early as possible; v-projection, later-pair
q/k proj+RoPE, AV of the previous pair and gating all ride as PE fillers
between score groups. DMA triggers are placed so no trigger ever carries a
buffer-reuse wait that would block an engine queue. Dummy matmuls at kernel
start keep the PE HAM clock-gate warm.
"""

import sys

if "/opt/trn_rl_repo" not in sys.path:
    sys.path.insert(0, "/opt/trn_rl_repo")

import numpy as np

import concourse.bass as bass
import concourse.tile as tile
from concourse import bacc, mybir
from concourse.bass_utils import run_bass_kernel_spmd
from concourse.masks import make_identity

F32 = mybir.dt.float32
BF16 = mybir.dt.bfloat16
AF = mybir.ActivationFunctionType
ALU = mybir.AluOpType

B = 8
N = 1024
DIM = 512
HEADS = 8
DHEAD = 64
THETA = 10000.0
N_CORES = 8

NT = N // 128  # 8 row tiles
KO = DIM // 128  # 4 contraction chunks
NC_ = N // 512  # 2 n-chunks of 512
MT = 4  # head-pair tiles (2 heads x 64 dims = 128 partitions)


def _rope_tables():
    """cos2T/sinS2T [128, N]: rows p = (h%2)*64 + d; identical per head half.

    sinS2T row 2t   = -sin(n * invf[t])  (multiplies shuffled value q[2t+1])
    sinS2T row 2t+1 = +sin(n * invf[t])
    """
    inv_freq = 1.0 / (THETA ** (np.arange(0, DHEAD, 2, dtype=np.float64) / DHEAD))
    pos = np.arange(N, dtype=np.float64)
    freqs = pos[None, :] * np.repeat(inv_freq, 2)[:, None]  # [64, N]
    cos = np.cos(freqs)
    sin = np.sin(freqs)
    sign = np.where(np.arange(DHEAD) % 2 == 0, -1.0, 1.0)[:, None]
    sin_signed = sin * sign
    cos2 = np.concatenate([cos, cos], axis=0)
    sin2 = np.concatenate([sin_signed, sin_signed], axis=0)
    return cos2, sin2


def build_kernel():
    nc = bacc.Bacc("TRN2", target_bir_lowering=False, debug=False, num_devices=N_CORES)

    x_d = nc.dram_tensor("x", [N, DIM], F32, kind="ExternalInput").ap()
    gamma_d = nc.dram_tensor("gamma", [DIM], F32, kind="ExternalInput").ap()
    wqkv_d = nc.dram_tensor("w_qkv", [DIM, 3 * DIM], F32, kind="ExternalInput").ap()
    wg_d = nc.dram_tensor("w_g", [DIM, HEADS], F32, kind="ExternalInput").ap()
    bg_d = nc.dram_tensor("b_g", [HEADS], F32, kind="ExternalInput").ap()
    wo_d = nc.dram_tensor("w_o", [DIM, DIM], F32, kind="ExternalInput").ap()
    out_d = nc.dram_tensor("out", [N, DIM], F32, kind="ExternalOutput").ap()

    import ml_dtypes

    cos_np, sin_np = _rope_tables()
    cos_d = nc.inline_tensor(
        cos_np.astype(ml_dtypes.bfloat16), name="rope_cos"
    ).ap()
    sin_d = nc.inline_tensor(
        sin_np.astype(ml_dtypes.bfloat16), name="rope_sin"
    ).ap()

    # E[h, mt*128 + p] = 1 if head h owns partition p of pair-tile mt
    e_np = np.zeros((HEADS, MT * 128), np.float32)
    for mt in range(MT):
        for p in range(128):
            e_np[2 * mt + p // 64, mt * 128 + p] = 1.0
    e_d = nc.inline_tensor(e_np.astype(ml_dtypes.bfloat16), name="gate_bcast_e").ap()

    with tile.TileContext(nc) as tc:
        _build_tile(nc, tc, x_d, gamma_d, wqkv_d, wg_d, bg_d, wo_d, cos_d, sin_d, e_d, out_d)

    nc.compile()
    return nc


def _build_tile(nc, tc, x_d, gamma_d, wqkv_d, wg_d, bg_d, wo_d, cos_d, sin_d, e_d, out_d):
    from contextlib import ExitStack

    ctx = ExitStack()
    with ctx:
        singles = ctx.enter_context(tc.tile_pool(name="singles", bufs=1))
        wpool = ctx.enter_context(tc.tile_pool(name="weights", bufs=1))
        xpool = ctx.enter_context(tc.tile_pool(name="x", bufs=3))
        xtp = ctx.enter_context(tc.tile_pool(name="xhatT", bufs=1))
        qkpool = ctx.enter_context(tc.tile_pool(name="qk", bufs=1))
        vpool = ctx.enter_context(tc.tile_pool(name="vaug", bufs=1))
        spool = ctx.enter_context(tc.tile_pool(name="expS", bufs=24))
        gpool = ctx.enter_context(tc.tile_pool(name="gates", bufs=1))
        avpool = ctx.enter_context(tc.tile_pool(name="avg", bufs=1))
        scratch = ctx.enter_context(tc.tile_pool(name="scratch", bufs=2))

        # ---- HBM loads; queue order == program order per engine ----
        # sync queue: x tiles (needed first), later wv/wo
        xt_tiles = []
        for it in range(NT):
            xt = xpool.tile([128, DIM], F32, tag="xt", name="xt")
            nc.sync.dma_start(out=xt[:], in_=x_d[it * 128:(it + 1) * 128, :])
            xt_tiles.append(xt)

        # scalar queue: q/k weight slices + rope tables (pair-0 critical path)
        wqk_stage = [
            scratch.tile([128, 2 * DIM], F32, tag=f"wqks{ko}", name=f"wqks{ko}",
                         bufs=1)
            for ko in range(KO)
        ]
        for ko in range(KO):
            nc.scalar.dma_start(
                out=wqk_stage[ko][:],
                in_=wqkv_d[ko * 128:(ko + 1) * 128, 0:2 * DIM],
            )
        cosT = singles.tile([128, N], BF16)
        sinT = singles.tile([128, N], BF16)
        nc.scalar.dma_start(out=cosT[:], in_=cos_d[:])
        nc.scalar.dma_start(out=sinT[:], in_=sin_d[:])

        # sync queue: small constants, then v/o weights (needed later)
        gamma_sb = singles.tile([128, KO], F32)
        nc.sync.dma_start(
            out=gamma_sb[:], in_=gamma_d.rearrange("(ko ki) -> ki ko", ki=128)
        )
        bg_sb = singles.tile([HEADS, 1], F32)
        nc.sync.dma_start(out=bg_sb[:], in_=bg_d.rearrange("(h o) -> h o", o=1))
        e_sb = singles.tile([HEADS, MT * 128], BF16, name="e_sb")
        nc.sync.dma_start(out=e_sb[:], in_=e_d[:])
        wg_stage = scratch.tile([128, KO * HEADS], F32, tag="wgs", name="wgs", bufs=1)
        for ko in range(KO):
            nc.sync.dma_start(
                out=wg_stage[:, ko * HEADS:(ko + 1) * HEADS],
                in_=wg_d[ko * 128:(ko + 1) * 128, :],
            )
        wv_stage = [
            scratch.tile([128, DIM], F32, tag=f"wvs{ko}", name=f"wvs{ko}", bufs=1)
            for ko in range(KO)
        ]
        for ko in range(KO):
            nc.sync.dma_start(
                out=wv_stage[ko][:],
                in_=wqkv_d[ko * 128:(ko + 1) * 128, 2 * DIM:3 * DIM],
            )
        wo_stage = [
            scratch.tile([128, DIM], F32, tag=f"wos{ko}", name=f"wos{ko}", bufs=1)
            for ko in range(KO)
        ]
        for ko in range(KO):
            nc.sync.dma_start(
                out=wo_stage[ko][:], in_=wo_d[ko * 128:(ko + 1) * 128, :]
            )

        # ---- HAM warmup: keep the PE clock-gate busy while x streams in ----
        zwu = singles.tile([128, 256], BF16, name="zwu")
        nc.gpsimd.memset(zwu[:], 0.0)
        ident = singles.tile([128, 128], BF16)
        make_identity(nc, ident)

        wqkv_sb = wpool.tile([128, KO, 3 * DIM], BF16)
        wg_sb = wpool.tile([128, KO, HEADS], BF16)
        wo_sb = wpool.tile([128, KO, DIM], BF16)

        xhatT = [
            xtp.tile([128, N], BF16, tag=f"xhatT{ko}", name=f"xhatT{ko}")
            for ko in range(KO)
        ]
        v_aug = [
            vpool.tile([128, HEADS * 128], BF16, tag=f"va{it}", name=f"va{it}")
            for it in range(NT)
        ]

        with tc.tile_pool(name="ps_wu", bufs=1, space="PSUM") as ps_wu:
            trash = ps_wu.tile([128, 512], F32, tag="trash", name="trash")
            for _ in range(40):
                nc.tensor.matmul(
                    trash[0:64, 0:256], zwu[:, 0:64], zwu[:], start=True, stop=True
                )

            # ---- RMS-normalize rows -> bf16 xhat (rides behind the x DMAs) ----
            xhat = []
            for it in range(NT):
                xt = xt_tiles[it]
                ss = scratch.tile([128, 1], F32, tag="ss", name="ss")
                nc.scalar.activation(
                    out=trash[:, :], in_=xt[:], func=AF.Square, accum_out=ss[:]
                )
                nc.scalar.activation(out=ss[:], in_=ss[:], func=AF.Sqrt, scale=1.0 / DIM)
                sinv = scratch.tile([128, 1], F32, tag="sinv", name="sinv")
                nc.vector.reciprocal(out=sinv[:], in_=ss[:])
                xtb = xpool.tile([128, DIM], BF16, tag="xtb", name="xtb")
                if it % 2 == 0:
                    nc.vector.tensor_scalar_mul(out=xtb[:], in0=xt[:], scalar1=sinv[:])
                else:
                    nc.scalar.activation(
                        out=xtb[:], in_=xt[:], func=AF.Copy, scale=sinv[:, 0:1]
                    )
                xhat.append(xtb)

            # fold gamma into weights, cast to bf16 (gpsimd, startup only)
            for ko in range(KO):
                nc.gpsimd.tensor_scalar_mul(
                    out=wqkv_sb[:, ko, 0:2 * DIM], in0=wqk_stage[ko][:],
                    scalar1=gamma_sb[:, ko:ko + 1],
                )
                nc.gpsimd.tensor_scalar_mul(
                    out=wg_sb[:, ko, :], in0=wg_stage[:, ko * HEADS:(ko + 1) * HEADS],
                    scalar1=gamma_sb[:, ko:ko + 1],
                )

            # ---- transpose xhat -> xhatT [dim(4x128), n] (bf16) ----
            with tc.tile_pool(name="ps_tr", bufs=2, space="PSUM") as ps_tr:
                for ic in range(NC_):
                    trps = ps_tr.tile([128, KO, 512], BF16, tag="trps", name="trps")
                    for s in range(4):
                        it = ic * 4 + s
                        for ko in range(KO):
                            nc.tensor.transpose(
                                trps[:, ko, s * 128:(s + 1) * 128],
                                xhat[it][:, ko * 128:(ko + 1) * 128],
                                ident[:],
                            )
                    for ko in range(KO):
                        nc.scalar.copy(
                            out=xhatT[ko][:, ic * 512:(ic + 1) * 512], in_=trps[:, ko, :]
                        )

            # v/o weights + v_aug ones (gpsimd, before the attention stream)
            for ko in range(KO):
                nc.gpsimd.tensor_scalar_mul(
                    out=wqkv_sb[:, ko, 2 * DIM:3 * DIM], in0=wv_stage[ko][:],
                    scalar1=gamma_sb[:, ko:ko + 1],
                )
                nc.gpsimd.tensor_copy(out=wo_sb[:, ko, :], in_=wo_stage[ko][:])
            for it in range(NT):
                va = v_aug[it][:].rearrange("p (q c) -> p q c", q=HEADS // 2)
                nc.gpsimd.memset(va[:, :, 64:192], 1.0)

        gTt = gpool.tile([HEADS, N], F32)
        effdent = gpool.tile([HEADS, N], F32)
        gT = gTt[:, :]
        effden = effdent[:, :]
        denomT = gpool.tile([HEADS, N], BF16)
        cTf = gpool.tile([HEADS, N], F32)
        cT = gpool.tile([HEADS, N], BF16)
        nbg = gpool.tile([HEADS, 1], F32)
        nc.gpsimd.tensor_scalar_mul(out=nbg[:], in0=bg_sb[:], scalar1=-1.0)
        nc.gpsimd.memset(denomT[:], 1.0)

        # gates: g_ps [8, N] = (w_g*gamma)^T @ xhatT; gT = exp(-g_ps - b_g)
        with tc.tile_pool(name="ps_g", bufs=1, space="PSUM") as ps_g:
            g_ps = ps_g.tile([HEADS, N], F32, tag="gps", name="gps", bufs=1)
            for ic in range(NC_):
                for ko in range(KO):
                    nc.tensor.matmul(
                        g_ps[:, ic * 512:(ic + 1) * 512],
                        wg_sb[:, ko, :],
                        xhatT[ko][:, ic * 512:(ic + 1) * 512],
                        start=(ko == 0),
                        stop=(ko == KO - 1),
                    )
            nc.scalar.activation(
                out=gT[:], in_=g_ps[:], func=AF.Exp, bias=nbg[:, 0:1], scale=-1.0
            )

        qT = [
            qkpool.tile([128, N], BF16, tag=f"q{mt}", name=f"q{mt}") for mt in range(MT)
        ]
        kT = [
            qkpool.tile([128, N], BF16, tag=f"k{mt}", name=f"k{mt}") for mt in range(MT)
        ]
        avg = [
            avpool.tile([128, N], BF16, tag=f"avg{mt}", name=f"avg{mt}")
            for mt in range(MT)
        ]
        shuf_mask = [(i ^ 1) for i in range(32)]
        scale = 1.0 / float(np.sqrt(DHEAD))

        with (
            tc.tile_pool(name="ps_p", bufs=2, space="PSUM") as ps_p,
            tc.tile_pool(name="ps_s", bufs=2, space="PSUM") as ps_s,
            tc.tile_pool(name="ps_av", bufs=2, space="PSUM") as ps_av,
        ):
            def mm(out_ps, lhsT, rhs, start, stop):
                nc.tensor.matmul(out_ps, lhsT, rhs, start=start, stop=stop)

            def v_chunk(it):
                """Project v row-tile it into the per-head v/ones lhsT blocks."""
                vps = ps_p.tile([128, 512], F32, tag="pps", name="vps")
                for ko in range(KO):
                    mm(
                        vps[:],
                        xhatT[ko][:, it * 128:(it + 1) * 128],
                        wqkv_sb[:, ko, 2 * DIM:3 * DIM],
                        start=(ko == 0),
                        stop=(ko == KO - 1),
                    )
                va = v_aug[it][:].rearrange("p (q c) -> p q c", q=HEADS // 2)
                vsrc = vps[:].rearrange("p (h c) -> p h c", h=HEADS)
                nc.vector.tensor_copy(out=va[:, :, 0:64], in_=vsrc[:, 0::2, :])
                nc.vector.tensor_copy(out=va[:, :, 192:256], in_=vsrc[:, 1::2, :])

            def qk_chunk(mt, which, ic):
                """Project+RoPE one [128, 512] chunk of qT/kT for pair mt."""
                dest = qT if which == 0 else kT
                col0 = which * DIM + mt * 128
                nslice = slice(ic * 512, (ic + 1) * 512)
                pps = ps_p.tile([128, 512], F32, tag="pps", name="pps")
                for ko in range(KO):
                    mm(
                        pps[:],
                        wqkv_sb[:, ko, col0:col0 + 128],
                        xhatT[ko][:, nslice],
                        start=(ko == 0),
                        stop=(ko == KO - 1),
                    )
                shuf = scratch.tile([128, 512], F32, tag="shuf", name="shuf", bufs=3)
                nc.vector.stream_shuffle(shuf[:], pps[:], mask=shuf_mask)
                t1 = scratch.tile([128, 512], BF16, tag="rt1", name="rt1", bufs=3)
                nc.vector.tensor_tensor(
                    out=t1[:], in0=pps[:], in1=cosT[:, nslice], op=ALU.mult
                )
                t2 = scratch.tile([128, 512], BF16, tag="rt2", name="rt2", bufs=3)
                nc.gpsimd.tensor_tensor(
                    out=t2[:], in0=shuf[:], in1=sinT[:, nslice], op=ALU.mult
                )
                nc.vector.tensor_tensor(
                    out=dest[mt][:, nslice], in0=t1[:], in1=t2[:], op=ALU.add
                )

            # per-pair post-attention state
            avraws = {}

            def av_group(mt, ic, hh, ess):
                """One accumulated AV^T matmul group + its avraw eviction."""
                h = 2 * mt + hh
                base = (h // 2) * 256 + (h % 2) * 128
                avp = ps_av.tile([128, 512], F32, tag="avp", name="avp")
                for jc in range(NT):
                    mm(
                        avp[:],
                        v_aug[jc][:, base:base + 128],
                        ess[jc][hh][:, ic * 512:(ic + 1) * 512],
                        start=(jc == 0),
                        stop=(jc == NT - 1),
                    )
                avraw = avraws[mt]
                nc.vector.tensor_copy(
                    out=avraw[:, hh * N + ic * 512:hh * N + (ic + 1) * 512],
                    in_=avp[:],
                )

            def pair_post(mt):
                """denominators -> cT -> cb broadcast -> gated avg tiles."""
                avraw = avraws[mt]
                h0 = 2 * mt
                nc.sync.dma_start(out=denomT[h0:h0 + 1, :], in_=avraw[64:65, 0:N])
                nc.sync.dma_start(
                    out=denomT[h0 + 1:h0 + 2, :], in_=avraw[0:1, N:2 * N]
                )
                # eff_denom = denom * (1 + gT);  cT = 1 / eff_denom
                nc.vector.scalar_tensor_tensor(
                    out=effden[:], in0=gT[:], scalar=1.0, in1=denomT[:],
                    op0=ALU.add, op1=ALU.mult,
                )
                nc.vector.reciprocal_approx_fast(out=cTf[:], in_=effden[:])
                nc.gpsimd.tensor_copy(out=cT[:], in_=cTf[:])
                for ic in range(NC_):
                    cb_ps = ps_p.tile([128, 512], F32, tag="pps", name="cbps")
                    mm(
                        cb_ps[:],
                        e_sb[:, mt * 128:(mt + 1) * 128],
                        cT[:, ic * 512:(ic + 1) * 512],
                        start=True,
                        stop=True,
                    )
                    isl = slice(ic * 512, (ic + 1) * 512)
                    nc.vector.tensor_tensor(
                        out=avg[mt][0:64, isl],
                        in0=avraw[0:64, ic * 512:(ic + 1) * 512],
                        in1=cb_ps[0:64, :],
                        op=ALU.mult,
                    )
                    nc.vector.tensor_tensor(
                        out=avg[mt][64:128, isl],
                        in0=avraw[64:128, N + ic * 512:N + (ic + 1) * 512],
                        in1=cb_ps[64:128, :],
                        op=ALU.mult,
                    )

            def attention_pair(mt, fillers):
                """Scores+exp stream for pair mt with PE fillers interleaved."""
                avraws[mt] = avpool.tile(
                    [128, 2 * N], BF16, tag="avraw", name="avraw", bufs=2
                )
                fillers = list(fillers)
                fi = 0
                ess = []
                for jc in range(NT):
                    pair = []
                    for hh in range(2):
                        pr = slice(hh * 64, (hh + 1) * 64)
                        sp = ps_s.tile([128, N], F32, tag="sps", name="sps")
                        for ic in range(NC_):
                            mm(
                                sp[:, ic * 512:(ic + 1) * 512],
                                kT[mt][pr, jc * 128:(jc + 1) * 128],
                                qT[mt][pr, ic * 512:(ic + 1) * 512],
                                start=True,
                                stop=True,
                            )
                        es = spool.tile([128, N], BF16, tag="es", name="es")
                        nc.scalar.activation(
                            out=es[:], in_=sp[:], func=AF.Exp, scale=scale
                        )
                        pair.append(es)
                    ess.append(pair)
                    # sprinkle independent PE work between score groups
                    take = 2 if jc >= 2 else 1
                    for _ in range(take):
                        if fi < len(fillers):
                            fillers[fi]()
                            fi += 1
                while fi < len(fillers):
                    fillers[fi]()
                    fi += 1
                return ess

            # ---- pair 0 prologue: q/k proj + RoPE as early as possible ----
            qk_chunk(0, 0, 0)
            qk_chunk(0, 0, 1)
            qk_chunk(0, 1, 0)
            qk_chunk(0, 1, 1)

            ess_by_mt = {}

            def make_av_fillers(mt):
                fills = []
                for ic in range(NC_):
                    for hh in range(2):
                        fills.append(
                            lambda mt=mt, ic=ic, hh=hh: av_group(
                                mt, ic, hh, ess_by_mt[mt]
                            )
                        )
                fills.append(lambda mt=mt: pair_post(mt))
                return fills

            # pair 0: fillers = v-projection (all 8 tiles) + pair-1 q/k proj
            f0 = [lambda it=it: v_chunk(it) for it in range(NT)]
            f0 += [lambda w=w, ic=ic: qk_chunk(1, w, ic) for w in (0, 1) for ic in range(NC_)]
            ess_by_mt[0] = attention_pair(0, f0)

            # pair 1: fillers = AV(0)+post(0) + pair-2 proj
            f1 = make_av_fillers(0)
            f1 += [lambda w=w, ic=ic: qk_chunk(2, w, ic) for w in (0, 1) for ic in range(NC_)]
            ess_by_mt[1] = attention_pair(1, f1)

            # pair 2: fillers = AV(1)+post(1) + pair-3 proj
            f2 = make_av_fillers(1)
            f2 += [lambda w=w, ic=ic: qk_chunk(3, w, ic) for w in (0, 1) for ic in range(NC_)]
            ess_by_mt[2] = attention_pair(2, f2)

            # pair 3: fillers = AV(2)+post(2)
            ess_by_mt[3] = attention_pair(3, make_av_fillers(2))

            # tail: AV(3)+post(3), then output projection
            for ic in range(NC_):
                for hh in range(2):
                    av_group(3, ic, hh, ess_by_mt[3])
            pair_post(3)

            # ---- output projection ----
            for it in range(NT):
                ops = ps_p.tile([128, DIM], F32, tag="pps", name="ops")
                for mt in range(MT):
                    mm(
                        ops[:],
                        avg[mt][:, it * 128:(it + 1) * 128],
                        wo_sb[:, mt, :],
                        start=(mt == 0),
                        stop=(mt == MT - 1),
                    )
                osb = scratch.tile([128, DIM], F32, tag="osb", name="osb")
                nc.scalar.copy(out=osb[:], in_=ops[:])
                nc.sync.dma_start(out=out_d[it * 128:(it + 1) * 128, :], in_=osb[:])


_NC_CACHE = None


def _get_nc():
    global _NC_CACHE
    if _NC_CACHE is None:
        _NC_CACHE = build_kernel()
    return _NC_CACHE


def kernel(**inputs):
    x = np.ascontiguousarray(np.asarray(inputs["x"], dtype=np.float32))
    gamma = np.ascontiguousarray(np.asarray(inputs["gamma"], dtype=np.float32))
    w_qkv = np.ascontiguousarray(np.asarray(inputs["w_qkv"], dtype=np.float32))
    w_g = np.ascontiguousarray(np.asarray(inputs["w_g"], dtype=np.float32))
    b_g = np.ascontiguousarray(np.asarray(inputs["b_g"], dtype=np.float32))
    w_o = np.ascontiguousarray(np.asarray(inputs["w_o"], dtype=np.float32))

    nc = _get_nc()
    in_maps = []
    for i in range(N_CORES):
        in_maps.append(
            {
                "x": np.ascontiguousarray(x[i]),
                "gamma": gamma,
                "w_qkv": w_qkv,
                "w_g": w_g,
                "b_g": b_g,
                "w_o": w_o,
            }
        )
    res = run_bass_kernel_spmd(nc, in_maps, core_ids=list(range(N_CORES)))
    out = np.stack([res.results[i]["out"] for i in range(N_CORES)], axis=0)
    return out.astype(np.float32)


if __name__ == "__main__":
    rng = np.random.default_rng(0)
    ins = {
        "x": rng.standard_normal((B, N, DIM), dtype=np.float32),
        "gamma": np.ones((DIM,), np.float32),
        "w_qkv": (rng.standard_normal((DIM, 3 * DIM), dtype=np.float32) / np.sqrt(DIM)),
        "w_g": (rng.standard_normal((DIM, HEADS), dtype=np.float32) / np.sqrt(DIM)),
        "b_g": np.zeros((HEADS,), np.float32),
        "w_o": (rng.standard_normal((DIM, DIM), dtype=np.float32) / np.sqrt(DIM)),
    }
    out = kernel(**ins)
    print("out", out.shape, out.dtype, float(np.abs(out).mean()))


# revision 22
# speedup vs baseline: 1.6563x; 1.6563x over previous
"""Gated multi-head attention (RMSNorm + RoPE + SDPA + sigmoid head gates + out-proj)
as a Trainium2 Bass/Tile kernel, data-parallel over batch across 8 NeuronCores.

Problem shapes (hardcoded): b=8, n=1024, dim=512, heads=8, dim_head=64, theta=1e4.
Each core processes one batch element; no collectives needed.

Per-core dataflow (bf16 matmul operands, fp32 PSUM accumulation):
  x [1024,512] --RMSnorm--> xhat(bf16) --PE transpose--> xhatT [512dim, 1024n]
  qT,kT = (w_qkv*gamma)^T-slices @ xhatT  (psum [128(2 heads x 64d), n])
  RoPE in transposed layout: q*cosT + stream_shuffle(q)*sinT_signed (bf16)
  v natural [n,512] -> v_aug lhsT blocks per head: 64 v cols + 64 ones cols
  (even heads v first, odd heads ones first so AV lands on home partitions)
  per head pair mt:
    S^T[j,i] = kT-slice @ qT-slice (K=64, one 1024-wide mm per (jc,hh),
    hh row-split on the PE), exp on ACT -> es[jc][hh] bf16 in SBUF;
    AV^T accumulated over j per 512-wide i chunk; ones columns give a
    replicated softmax denominator on the opposite partition half
  denominator rows -> denomT[8, n] via SBUF DMA; gate folded as
  eff_denom = denom * (1 + exp(-(xn@w_g + b_g))) so gate+normalize is one
  reciprocal: cT = 1/eff_denom (no Sigmoid table set on ACT)
  cb = E-matmul broadcast of cT to head partitions (bf16 psum, read directly
  by the avg multiplies); avg = avraw * cb; out[i,:] = sum_mt avg @ w_o-slice

Scheduling: the exp stream on ACT (64 x [128,1024]) is the pacing resource.
Pair-0 scores are issued as early as possible; v-projection, later-pair
q/k proj+RoPE, AV of the previous pair and gating all ride as PE fillers
between score groups. DMA triggers are placed so no trigger ever carries a
buffer-reuse wait that would block an engine queue. Dummy matmuls at kernel
start keep the PE HAM clock-gate warm.
"""

import sys

if "/opt/trn_rl_repo" not in sys.path:
    sys.path.insert(0, "/opt/trn_rl_repo")

import numpy as np

import concourse.bass as bass
import concourse.tile as tile
from concourse import bacc, mybir
from concourse.bass_utils import run_bass_kernel_spmd
from concourse.masks import make_identity

F32 = mybir.dt.float32
BF16 = mybir.dt.bfloat16
AF = mybir.ActivationFunctionType
ALU = mybir.AluOpType

B = 8
N = 1024
DIM = 512
HEADS = 8
DHEAD = 64
THETA = 10000.0
N_CORES = 8

NT = N // 128  # 8 row tiles
KO = DIM // 128  # 4 contraction chunks
NC_ = N // 512  # 2 n-chunks of 512
MT = 4  # head-pair tiles (2 heads x 64 dims = 128 partitions)


def _rope_tables():
    """cos2T/sinS2T [128, N]: rows p = (h%2)*64 + d; identical per head half.

    sinS2T row 2t   = -sin(n * invf[t])  (multiplies shuffled value q[2t+1])
    sinS2T row 2t+1 = +sin(n * invf[t])
    """
    inv_freq = 1.0 / (THETA ** (np.arange(0, DHEAD, 2, dtype=np.float64) / DHEAD))
    pos = np.arange(N, dtype=np.float64)
    freqs = pos[None, :] * np.repeat(inv_freq, 2)[:, None]  # [64, N]
    cos = np.cos(freqs)
    sin = np.sin(freqs)
    sign = np.where(np.arange(DHEAD) % 2 == 0, -1.0, 1.0)[:, None]
    sin_signed = sin * sign
    cos2 = np.concatenate([cos, cos], axis=0)
    sin2 = np.concatenate([sin_signed, sin_signed], axis=0)
    return cos2, sin2


def build_kernel():
    nc = bacc.Bacc("TRN2", target_bir_lowering=False, debug=False, num_devices=N_CORES)

    x_d = nc.dram_tensor("x", [N, DIM], F32, kind="ExternalInput").ap()
    gamma_d = nc.dram_tensor("gamma", [DIM], F32, kind="ExternalInput").ap()
    wqkv_d = nc.dram_tensor("w_qkv", [DIM, 3 * DIM], F32, kind="ExternalInput").ap()
    wg_d = nc.dram_tensor("w_g", [DIM, HEADS], F32, kind="ExternalInput").ap()
    bg_d = nc.dram_tensor("b_g", [HEADS], F32, kind="ExternalInput").ap()
    wo_d = nc.dram_tensor("w_o", [DIM, DIM], F32, kind="ExternalInput").ap()
    out_d = nc.dram_tensor("out", [N, DIM], F32, kind="ExternalOutput").ap()

    import ml_dtypes

    cos_np, sin_np = _rope_tables()
    cos_d = nc.inline_tensor(
        cos_np.astype(ml_dtypes.bfloat16), name="rope_cos"
    ).ap()
    sin_d = nc.inline_tensor(
        sin_np.astype(ml_dtypes.bfloat16), name="rope_sin"
    ).ap()

    # E[h, mt*128 + p] = 1 if head h owns partition p of pair-tile mt
    e_np = np.zeros((HEADS, MT * 128), np.float32)
    for mt in range(MT):
        for p in range(128):
            e_np[2 * mt + p // 64, mt * 128 + p] = 1.0
    e_d = nc.inline_tensor(e_np.astype(ml_dtypes.bfloat16), name="gate_bcast_e").ap()

    with tile.TileContext(nc) as tc:
        _build_tile(nc, tc, x_d, gamma_d, wqkv_d, wg_d, bg_d, wo_d, cos_d, sin_d, e_d, out_d)

    nc.compile()
    return nc


def _build_tile(nc, tc, x_d, gamma_d, wqkv_d, wg_d, bg_d, wo_d, cos_d, sin_d, e_d, out_d):
    from contextlib import ExitStack

    ctx = ExitStack()
    with ctx:
        singles = ctx.enter_context(tc.tile_pool(name="singles", bufs=1))
        wpool = ctx.enter_context(tc.tile_pool(name="weights", bufs=1))
        xpool = ctx.enter_context(tc.tile_pool(name="x", bufs=3))
        xtp = ctx.enter_context(tc.tile_pool(name="xhatT", bufs=1))
        qkpool = ctx.enter_context(tc.tile_pool(name="qk", bufs=1))
        vpool = ctx.enter_context(tc.tile_pool(name="vaug", bufs=1))
        spool = ctx.enter_context(tc.tile_pool(name="expS", bufs=24))
        gpool = ctx.enter_context(tc.tile_pool(name="gates", bufs=1))
        avpool = ctx.enter_context(tc.tile_pool(name="avg", bufs=1))
        scratch = ctx.enter_context(tc.tile_pool(name="scratch", bufs=2))

        # ---- HBM loads; queue order == program order per engine ----
        # sync queue: x tiles (needed first), later wv/wo
        xt_tiles = []
        for it in range(NT):
            xt = xpool.tile([128, DIM], F32, tag="xt", name="xt")
            nc.sync.dma_start(out=xt[:], in_=x_d[it * 128:(it + 1) * 128, :])
            xt_tiles.append(xt)

        # scalar queue: q/k weight slices + rope tables (pair-0 critical path)
        wqk_stage = [
            scratch.tile([128, 2 * DIM], F32, tag=f"wqks{ko}", name=f"wqks{ko}",
                         bufs=1)
            for ko in range(KO)
        ]
        for ko in range(KO):
            nc.scalar.dma_start(
                out=wqk_stage[ko][:],
                in_=wqkv_d[ko * 128:(ko + 1) * 128, 0:2 * DIM],
            )
        cosT = singles.tile([128, N], BF16)
        sinT = singles.tile([128, N], BF16)
        nc.scalar.dma_start(out=cosT[:], in_=cos_d[:])
        nc.scalar.dma_start(out=sinT[:], in_=sin_d[:])

        # sync queue: small constants, then v/o weights (needed later)
        gamma_sb = singles.tile([128, KO], F32)
        nc.sync.dma_start(
            out=gamma_sb[:], in_=gamma_d.rearrange("(ko ki) -> ki ko", ki=128)
        )
        bg_sb = singles.tile([HEADS, 1], F32)
        nc.sync.dma_start(out=bg_sb[:], in_=bg_d.rearrange("(h o) -> h o", o=1))
        e_sb = singles.tile([HEADS, MT * 128], BF16, name="e_sb")
        nc.sync.dma_start(out=e_sb[:], in_=e_d[:])
        wg_stage = scratch.tile([128, KO * HEADS], F32, tag="wgs", name="wgs", bufs=1)
        for ko in range(KO):
            nc.sync.dma_start(
                out=wg_stage[:, ko * HEADS:(ko + 1) * HEADS],
                in_=wg_d[ko * 128:(ko + 1) * 128, :],
            )
        wv_stage = [
            scratch.tile([128, DIM], F32, tag=f"wvs{ko}", name=f"wvs{ko}", bufs=1)
            for ko in range(KO)
        ]
        for ko in range(KO):
            nc.sync.dma_start(
                out=wv_stage[ko][:],
                in_=wqkv_d[ko * 128:(ko + 1) * 128, 2 * DIM:3 * DIM],
            )
        wo_stage = [
            scratch.tile([128, DIM], F32, tag=f"wos{ko}", name=f"wos{ko}", bufs=1)
            for ko in range(KO)
        ]
        for ko in range(KO):
            nc.sync.dma_start(
                out=wo_stage[ko][:], in_=wo_d[ko * 128:(ko + 1) * 128, :]
            )

        # ---- HAM warmup: keep the PE clock-gate busy while x streams in ----
        zwu = singles.tile([128, 256], BF16, name="zwu")
        nc.gpsimd.memset(zwu[:], 0.0)
        ident = singles.tile([128, 128], BF16)
        make_identity(nc, ident)

        wqkv_sb = wpool.tile([128, KO, 3 * DIM], BF16)
        wg_sb = wpool.tile([128, KO, HEADS], BF16)
        wo_sb = wpool.tile([128, KO, DIM], BF16)

        xhatT = [
            xtp.tile([128, N], BF16, tag=f"xhatT{ko}", name=f"xhatT{ko}")
            for ko in range(KO)
        ]
        v_aug = [
            vpool.tile([128, HEADS * 128], BF16, tag=f"va{it}", name=f"va{it}")
            for it in range(NT)
        ]

        with tc.tile_pool(name="ps_wu", bufs=1, space="PSUM") as ps_wu:
            trash = ps_wu.tile([128, 512], F32, tag="trash", name="trash")
            for _ in range(40):
                nc.tensor.matmul(
                    trash[0:64, 0:256], zwu[:, 0:64], zwu[:], start=True, stop=True
                )

            # ---- RMS-normalize rows -> bf16 xhat (rides behind the x DMAs) ----
            xhat = []
            for it in range(NT):
                xt = xt_tiles[it]
                ss = scratch.tile([128, 1], F32, tag="ss", name="ss")
                nc.scalar.activation(
                    out=trash[:, :], in_=xt[:], func=AF.Square, accum_out=ss[:]
                )
                nc.scalar.activation(out=ss[:], in_=ss[:], func=AF.Sqrt, scale=1.0 / DIM)
                sinv = scratch.tile([128, 1], F32, tag="sinv", name="sinv")
                nc.vector.reciprocal(out=sinv[:], in_=ss[:])
                xtb = xpool.tile([128, DIM], BF16, tag="xtb", name="xtb")
                nc.vector.tensor_scalar_mul(out=xtb[:], in0=xt[:], scalar1=sinv[:])
                xhat.append(xtb)

            # fold gamma into weights, cast to bf16 (DVE; TensorScalar ops are
            # ~20x slower on gpsimd, keep them off that engine)
            for ko in range(KO):
                nc.vector.tensor_scalar_mul(
                    out=wqkv_sb[:, ko, 0:2 * DIM], in0=wqk_stage[ko][:],
                    scalar1=gamma_sb[:, ko:ko + 1],
                )
                nc.vector.tensor_scalar_mul(
                    out=wg_sb[:, ko, :], in0=wg_stage[:, ko * HEADS:(ko + 1) * HEADS],
                    scalar1=gamma_sb[:, ko:ko + 1],
                )

            # ---- transpose xhat -> xhatT [dim(4x128), n] (bf16) ----
            with tc.tile_pool(name="ps_tr", bufs=2, space="PSUM") as ps_tr:
                for ic in range(NC_):
                    trps = ps_tr.tile([128, KO, 512], BF16, tag="trps", name="trps")
                    for s in range(4):
                        it = ic * 4 + s
                        for ko in range(KO):
                            nc.tensor.transpose(
                                trps[:, ko, s * 128:(s + 1) * 128],
                                xhat[it][:, ko * 128:(ko + 1) * 128],
                                ident[:],
                            )
                    for ko in range(KO):
                        nc.scalar.copy(
                            out=xhatT[ko][:, ic * 512:(ic + 1) * 512], in_=trps[:, ko, :]
                        )

            # v/o weights + v_aug ones (before the attention stream)
            for ko in range(KO):
                nc.vector.tensor_scalar_mul(
                    out=wqkv_sb[:, ko, 2 * DIM:3 * DIM], in0=wv_stage[ko][:],
                    scalar1=gamma_sb[:, ko:ko + 1],
                )
                nc.vector.tensor_copy(out=wo_sb[:, ko, :], in_=wo_stage[ko][:])
            for it in range(NT):
                va = v_aug[it][:].rearrange("p (q c) -> p q c", q=HEADS // 2)
                nc.gpsimd.memset(va[:, :, 64:192], 1.0)

        gTt = gpool.tile([HEADS, N], F32)
        effdent = gpool.tile([HEADS, N], F32)
        gT = gTt[:, :]
        effden = effdent[:, :]
        denomT = gpool.tile([HEADS, N], BF16)
        cTf = gpool.tile([HEADS, N], F32)
        cT = gpool.tile([HEADS, N], BF16)
        nbg = gpool.tile([HEADS, 1], F32)
        nc.vector.tensor_scalar_mul(out=nbg[:], in0=bg_sb[:], scalar1=-1.0)
        nc.gpsimd.memset(denomT[:], 1.0)

        # gates: g_ps [8, N] = (w_g*gamma)^T @ xhatT; gT = exp(-g_ps - b_g)
        with tc.tile_pool(name="ps_g", bufs=1, space="PSUM") as ps_g:
            g_ps = ps_g.tile([HEADS, N], F32, tag="gps", name="gps", bufs=1)
            for ic in range(NC_):
                for ko in range(KO):
                    nc.tensor.matmul(
                        g_ps[:, ic * 512:(ic + 1) * 512],
                        wg_sb[:, ko, :],
                        xhatT[ko][:, ic * 512:(ic + 1) * 512],
                        start=(ko == 0),
                        stop=(ko == KO - 1),
                    )
            nc.scalar.activation(
                out=gT[:], in_=g_ps[:], func=AF.Exp, bias=nbg[:, 0:1], scale=-1.0
            )

        qT = [
            qkpool.tile([128, N], BF16, tag=f"q{mt}", name=f"q{mt}") for mt in range(MT)
        ]
        kT = [
            qkpool.tile([128, N], BF16, tag=f"k{mt}", name=f"k{mt}") for mt in range(MT)
        ]
        avg = [
            avpool.tile([128, N], BF16, tag=f"avg{mt}", name=f"avg{mt}")
            for mt in range(MT)
        ]
        shuf_mask = [(i ^ 1) for i in range(32)]
        scale = 1.0 / float(np.sqrt(DHEAD))

        with (
            tc.tile_pool(name="ps_p", bufs=2, space="PSUM") as ps_p,
            tc.tile_pool(name="ps_s", bufs=2, space="PSUM") as ps_s,
            tc.tile_pool(name="ps_av", bufs=2, space="PSUM") as ps_av,
        ):
            def mm(out_ps, lhsT, rhs, start, stop):
                nc.tensor.matmul(out_ps, lhsT, rhs, start=start, stop=stop)

            def v_chunk(it):
                """Project v row-tile it into the per-head v/ones lhsT blocks."""
                vps = ps_p.tile([128, 512], F32, tag="pps", name="vps")
                for ko in range(KO):
                    mm(
                        vps[:],
                        xhatT[ko][:, it * 128:(it + 1) * 128],
                        wqkv_sb[:, ko, 2 * DIM:3 * DIM],
                        start=(ko == 0),
                        stop=(ko == KO - 1),
                    )
                va = v_aug[it][:].rearrange("p (q c) -> p q c", q=HEADS // 2)
                vsrc = vps[:].rearrange("p (h c) -> p h c", h=HEADS)
                nc.vector.tensor_copy(out=va[:, :, 0:64], in_=vsrc[:, 0::2, :])
                nc.vector.tensor_copy(out=va[:, :, 192:256], in_=vsrc[:, 1::2, :])

            def qk_chunk(mt, which, ic):
                """Project+RoPE one [128, 512] chunk of qT/kT for pair mt."""
                dest = qT if which == 0 else kT
                col0 = which * DIM + mt * 128
                nslice = slice(ic * 512, (ic + 1) * 512)
                pps = ps_p.tile([128, 512], F32, tag="pps", name="pps")
                for ko in range(KO):
                    mm(
                        pps[:],
                        wqkv_sb[:, ko, col0:col0 + 128],
                        xhatT[ko][:, nslice],
                        start=(ko == 0),
                        stop=(ko == KO - 1),
                    )
                shuf = scratch.tile([128, 512], F32, tag="shuf", name="shuf", bufs=3)
                nc.vector.stream_shuffle(shuf[:], pps[:], mask=shuf_mask)
                t1 = scratch.tile([128, 512], BF16, tag="rt1", name="rt1", bufs=3)
                nc.vector.tensor_tensor(
                    out=t1[:], in0=pps[:], in1=cosT[:, nslice], op=ALU.mult
                )
                t2 = scratch.tile([128, 512], BF16, tag="rt2", name="rt2", bufs=3)
                nc.gpsimd.tensor_tensor(
                    out=t2[:], in0=shuf[:], in1=sinT[:, nslice], op=ALU.mult
                )
                nc.gpsimd.tensor_tensor(
                    out=dest[mt][:, nslice], in0=t1[:], in1=t2[:], op=ALU.add
                )

            # per-pair post-attention state
            avraws = {}

            def av_group(mt, ic, hh, ess):
                """One accumulated AV^T matmul group + its avraw eviction."""
                h = 2 * mt + hh
                base = (h // 2) * 256 + (h % 2) * 128
                avp = ps_av.tile([128, 512], F32, tag="avp", name="avp")
                for jc in range(NT):
                    mm(
                        avp[:],
                        v_aug[jc][:, base:base + 128],
                        ess[jc][hh][:, ic * 512:(ic + 1) * 512],
                        start=(jc == 0),
                        stop=(jc == NT - 1),
                    )
                avraw = avraws[mt]
                nc.vector.tensor_copy(
                    out=avraw[:, hh * N + ic * 512:hh * N + (ic + 1) * 512],
                    in_=avp[:],
                )

            def pair_post(mt):
                """denominators -> cT -> cb broadcast -> gated avg tiles."""
                avraw = avraws[mt]
                h0 = 2 * mt
                nc.sync.dma_start(out=denomT[h0:h0 + 1, :], in_=avraw[64:65, 0:N])
                nc.sync.dma_start(
                    out=denomT[h0 + 1:h0 + 2, :], in_=avraw[0:1, N:2 * N]
                )
                # eff_denom = denom * (1 + gT);  cT = 1 / eff_denom
                nc.vector.scalar_tensor_tensor(
                    out=effden[:], in0=gT[:], scalar=1.0, in1=denomT[:],
                    op0=ALU.add, op1=ALU.mult,
                )
                nc.vector.reciprocal_approx_fast(out=cTf[:], in_=effden[:])
                nc.vector.tensor_copy(out=cT[:], in_=cTf[:])
                for ic in range(NC_):
                    cb_ps = ps_p.tile([128, 512], F32, tag="pps", name="cbps")
                    mm(
                        cb_ps[:],
                        e_sb[:, mt * 128:(mt + 1) * 128],
                        cT[:, ic * 512:(ic + 1) * 512],
                        start=True,
                        stop=True,
                    )
                    isl = slice(ic * 512, (ic + 1) * 512)
                    nc.vector.tensor_tensor(
                        out=avg[mt][0:64, isl],
                        in0=avraw[0:64, ic * 512:(ic + 1) * 512],
                        in1=cb_ps[0:64, :],
                        op=ALU.mult,
                    )
                    nc.vector.tensor_tensor(
                        out=avg[mt][64:128, isl],
                        in0=avraw[64:128, N + ic * 512:N + (ic + 1) * 512],
                        in1=cb_ps[64:128, :],
                        op=ALU.mult,
                    )

            def attention_pair(mt, fillers):
                """Scores+exp stream for pair mt with PE fillers interleaved."""
                avraws[mt] = avpool.tile(
                    [128, 2 * N], BF16, tag="avraw", name="avraw", bufs=2
                )
                fillers = list(fillers)
                fi = 0
                ess = []
                for jc in range(NT):
                    pair = []
                    for hh in range(2):
                        pr = slice(hh * 64, (hh + 1) * 64)
                        sp = ps_s.tile([128, N], F32, tag="sps", name="sps")
                        for ic in range(NC_):
                            mm(
                                sp[:, ic * 512:(ic + 1) * 512],
                                kT[mt][pr, jc * 128:(jc + 1) * 128],
                                qT[mt][pr, ic * 512:(ic + 1) * 512],
                                start=True,
                                stop=True,
                            )
                        es = spool.tile([128, N], BF16, tag="es", name="es")
                        nc.scalar.activation(
                            out=es[:], in_=sp[:], func=AF.Exp, scale=scale
                        )
                        pair.append(es)
                    ess.append(pair)
                    # sprinkle independent PE work between score groups
                    take = 2 if jc >= 2 else 1
                    for _ in range(take):
                        if fi < len(fillers):
                            fillers[fi]()
                            fi += 1
                while fi < len(fillers):
                    fillers[fi]()
                    fi += 1
                return ess

            # ---- pair 0 prologue: q/k proj + RoPE as early as possible ----
            qk_chunk(0, 0, 0)
            qk_chunk(0, 0, 1)
            qk_chunk(0, 1, 0)
            qk_chunk(0, 1, 1)

            ess_by_mt = {}

            def make_av_fillers(mt):
                fills = []
                for ic in range(NC_):
                    for hh in range(2):
                        fills.append(
                            lambda mt=mt, ic=ic, hh=hh: av_group(
                                mt, ic, hh, ess_by_mt[mt]
                            )
                        )
                fills.append(lambda mt=mt: pair_post(mt))
                return fills

            # pair 0: fillers = v-projection (all 8 tiles) + pair-1 q/k proj
            f0 = [lambda it=it: v_chunk(it) for it in range(NT)]
            f0 += [lambda w=w, ic=ic: qk_chunk(1, w, ic) for w in (0, 1) for ic in range(NC_)]
            ess_by_mt[0] = attention_pair(0, f0)

            # pair 1: fillers = AV(0)+post(0) + pair-2 proj
            f1 = make_av_fillers(0)
            f1 += [lambda w=w, ic=ic: qk_chunk(2, w, ic) for w in (0, 1) for ic in range(NC_)]
            ess_by_mt[1] = attention_pair(1, f1)

            # pair 2: fillers = AV(1)+post(1) + pair-3 proj
            f2 = make_av_fillers(1)
            f2 += [lambda w=w, ic=ic: qk_chunk(3, w, ic) for w in (0, 1) for ic in range(NC_)]
            ess_by_mt[2] = attention_pair(2, f2)

            # pair 3: fillers = AV(2)+post(2)
            ess_by_mt[3] = attention_pair(3, make_av_fillers(2))

            # tail: AV(3)+post(3), then output projection
            for ic in range(NC_):
                for hh in range(2):
                    av_group(3, ic, hh, ess_by_mt[3])
            pair_post(3)

            # ---- output projection ----
            for it in range(NT):
                ops = ps_p.tile([128, DIM], F32, tag="pps", name="ops")
                for mt in range(MT):
                    mm(
                        ops[:],
                        avg[mt][:, it * 128:(it + 1) * 128],
                        wo_sb[:, mt, :],
                        start=(mt == 0),
                        stop=(mt == MT - 1),
                    )
                osb = scratch.tile([128, DIM], F32, tag="osb", name="osb")
                nc.scalar.copy(out=osb[:], in_=ops[:])
                nc.sync.dma_start(out=out_d[it * 128:(it + 1) * 128, :], in_=osb[:])


_NC_CACHE = None


def _get_nc():
    global _NC_CACHE
    if _NC_CACHE is None:
        _NC_CACHE = build_kernel()
    return _NC_CACHE


def kernel(**inputs):
    x = np.ascontiguousarray(np.asarray(inputs["x"], dtype=np.float32))
    gamma = np.ascontiguousarray(np.asarray(inputs["gamma"], dtype=np.float32))
    w_qkv = np.ascontiguousarray(np.asarray(inputs["w_qkv"], dtype=np.float32))
    w_g = np.ascontiguousarray(np.asarray(inputs["w_g"], dtype=np.float32))
    b_g = np.ascontiguousarray(np.asarray(inputs["b_g"], dtype=np.float32))
    w_o = np.ascontiguousarray(np.asarray(inputs["w_o"], dtype=np.float32))

    nc = _get_nc()
    in_maps = []
    for i in range(N_CORES):
        in_maps.append(
            {
                "x": np.ascontiguousarray(x[i]),
                "gamma": gamma,
                "w_qkv": w_qkv,
                "w_g": w_g,
                "b_g": b_g,
                "w_o": w_o,
            }
        )
    res = run_bass_kernel_spmd(nc, in_maps, core_ids=list(range(N_CORES)))
    out = np.stack([res.results[i]["out"] for i in range(N_CORES)], axis=0)
    return out.astype(np.float32)


if __name__ == "__main__":
    rng = np.random.default_rng(0)
    ins = {
        "x": rng.standard_normal((B, N, DIM), dtype=np.float32),
        "gamma": np.ones((DIM,), np.float32),
        "w_qkv": (rng.standard_normal((DIM, 3 * DIM), dtype=np.float32) / np.sqrt(DIM)),
        "w_g": (rng.standard_normal((DIM, HEADS), dtype=np.float32) / np.sqrt(DIM)),
        "b_g": np.zeros((HEADS,), np.float32),
        "w_o": (rng.standard_normal((DIM, DIM), dtype=np.float32) / np.sqrt(DIM)),
    }
    out = kernel(**ins)
    print("out", out.shape, out.dtype, float(np.abs(out).mean()))
